# revision 16
# baseline (speedup 1.0000x reference)
"""Trainium2 Bass kernel for nn_BaseModel_18081812316555.

Pipeline per sample (data-parallel over batch, 2 samples/core x 8 cores):
  hpre = Wall @ x            (Wall folds electrode conv + channel expand +
                              constrained depthwise spatial conv + pointwise)
  xhat = LN1(hpre)           -> bf16
  hcv  = depthwise temporal conv (PE diag-matmul accumulation, bf16->fp32 PSUM)
  hcat = LN2(hcv)
  h_aT = (w_aff @ w_sconv) @ hcat, built transposed (t on partitions)
  P    = gram(h_aT)/(T-1) - S S^T/(T(T-1)) + 1e-4 w_aff w_aff^T
  L    = logm(P) via Chebyshev/Clenshaw matrix polynomial on [0.05, 5.8]
  out  = L * vec-scale mask  (host extracts triu + final 4-way FC)
"""

import numpy as np
import ml_dtypes

B, CN, T = 16, 22, 1000
EXP, FCH, SPD, NCLS = 300, 150, 100, 4
KS = (15, 75, 55)
NCORES = 8
SPC = B // NCORES            # samples per core
CHEB_DEG = 32
CHEB_LO, CHEB_HI = 0.05, 5.8
EPS_LN = 1e-5

# channel chunking for the temporal conv (450 = 3 blocks x 150)
CHUNKS = [(0, 128), (128, 128), (256, 128), (384, 66)]   # (start, height)
HALF = 500                   # time-half width (2 x 500 = T)
MARG = 37                    # max K//2 across blocks


def _chan_K(ch):
    return KS[ch // FCH]


def _chunk_taps(c0, h):
    k2 = max(_chan_K(c0 + i) // 2 for i in range(h))
    return list(range(-k2, k2 + 1))


def _host_prep(inputs):
    d = {k: np.asarray(v, np.float64) for k, v in inputs.items()}
    wall, w5s, g_triv = [], [], True
    for bi, K in zip((1, 2, 3), KS):
        w1, w2 = d[f'b{bi}_w1'], d[f'b{bi}_w2']
        w3, w4 = d[f'b{bi}_w3'], d[f'b{bi}_w4']
        n = np.linalg.norm(w3, axis=1, keepdims=True)
        w3r = w3 * np.minimum(1.0, 1.0 / (n + 1e-12))
        weff = w4 @ (w2[:, None] * w3r)                      # (150,22)
        C = np.zeros((CN, CN))
        for i in range(CN):
            for j, k in enumerate((-1, 0, 1)):
                if 0 <= i + k < CN:
                    C[i, i + k] = w1[j]
        wall.append(weff @ C)
        w5s.append(d[f'b{bi}_w5'][:, 0, :])                  # (150,K)
        for nm in ('g1', 'g2'):
            if not np.allclose(d[f'b{bi}_{nm}'], 1.0):
                g_triv = False
        for nm in ('b1', 'b2'):
            if not np.allclose(d[f'b{bi}_{nm}'], 0.0):
                g_triv = False
    if not g_triv:
        raise NotImplementedError("non-trivial LN gain/bias not supported")
    wall = np.concatenate(wall, 0)                           # (450,22)
    w5cat = np.zeros((3 * FCH, max(KS)))
    for i, K in enumerate(KS):
        w5cat[FCH * i:FCH * (i + 1), :K] = w5s[i]

    # diag strips, bf16: strip[p, u*h + q] = w(chan, delta_u) * (p == q)
    strips = []
    for c0, h in CHUNKS:
        taps = _chunk_taps(c0, h)
        s = np.zeros((h, len(taps) * h), np.float64)
        for u, dlt in enumerate(taps):
            for p in range(h):
                K = _chan_K(c0 + p)
                k = dlt + K // 2
                if 0 <= k < K:
                    s[p, u * h + p] = w5cat[c0 + p, k]
        strips.append(s.astype(ml_dtypes.bfloat16))

    w_sa = d['w_aff'] @ d['w_sconv']                         # (100,450)
    cheb = np.polynomial.chebyshev.chebinterpolate(
        lambda y: np.log((CHEB_LO + CHEB_HI) / 2 + y * (CHEB_HI - CHEB_LO) / 2),
        CHEB_DEG)
    iu = np.triu_indices(SPD)
    msc = np.full((SPD, SPD), np.sqrt(2.0))
    np.fill_diagonal(msc, 1.0)

    f32 = np.float32
    return dict(
        wallT=np.ascontiguousarray(wall.T).astype(ml_dtypes.bfloat16),
        strips=strips,
        wsaT=np.ascontiguousarray(w_sa.T, f32),              # (450,100)
        i100=np.eye(SPD, dtype=f32),
        ridge=(1e-4 * (d['w_aff'] @ d['w_aff'].T)).astype(f32),
        iy=(-(CHEB_LO + CHEB_HI) / (CHEB_HI - CHEB_LO) * np.eye(SPD)).astype(f32),
        mscale=msc.astype(f32),
        cheb=[float(c) for c in cheb],
        w_fc=d['w_fc'].astype(f32), b_fc=d['b_fc'].astype(f32), iu=iu,
    )


_PROG = None


def _cheb_coeffs():
    c = np.polynomial.chebyshev.chebinterpolate(
        lambda y: np.log((CHEB_LO + CHEB_HI) / 2 + y * (CHEB_HI - CHEB_LO) / 2),
        CHEB_DEG)
    return [float(np.float32(v)) for v in c]


def _build_program():
    import concourse.bacc as bacc
    import concourse.tile as tile
    import concourse.mybir as mybir
    import concourse.bass as bass

    f32 = mybir.dt.float32
    bf16 = mybir.dt.bfloat16
    AL = mybir.AluOpType
    AF = mybir.ActivationFunctionType
    PSUM = bass.MemorySpace.PSUM

    nc = bacc.Bacc("TRN2", target_bir_lowering=False, debug=False)

    x2_d = nc.dram_tensor("x2", [SPC, CN, T], bf16, kind="ExternalInput")
    wallT_d = nc.dram_tensor("wallT", [CN, 3 * FCH], bf16, kind="ExternalInput")
    strip_d = []
    for ci, (c0, h) in enumerate(CHUNKS):
        nt = len(_chunk_taps(c0, h))
        strip_d.append(nc.dram_tensor(f"strip{ci}", [h, nt * h], bf16,
                                      kind="ExternalInput"))
    wsaT_d = nc.dram_tensor("wsaT", [3 * FCH, SPD], f32, kind="ExternalInput")
    i100_d = nc.dram_tensor("i100", [SPD, SPD], f32, kind="ExternalInput")
    ridge_d = nc.dram_tensor("ridge", [SPD, SPD], f32, kind="ExternalInput")
    iy_d = nc.dram_tensor("iy", [SPD, SPD], f32, kind="ExternalInput")
    msc_d = nc.dram_tensor("mscale", [SPD, SPD], f32, kind="ExternalInput")
    lout_d = nc.dram_tensor("lout", [SPC, SPD, SPD], f32, kind="ExternalOutput")

    CC = _cheb_coeffs()
    NTT = 8          # time tiles for the transposed h_a (8 x 125 = 1000)
    TTW = T // NTT

    with tile.TileContext(nc) as tc:
        with (
            tc.tile_pool(name="const", bufs=1) as cpool,
            tc.tile_pool(name="work", bufs=1) as wpool,
            tc.tile_pool(name="small", bufs=2) as spool,
            tc.tile_pool(name="ps_big", bufs=4, space=PSUM) as ps_big,
            tc.tile_pool(name="ps_hat", bufs=2, space=PSUM) as ps_hat,
            tc.tile_pool(name="ps_sm", bufs=2, space=PSUM) as ps_sm,
        ):
            # ---- constants / weights into SBUF ----
            wallT = cpool.tile([CN, 3 * FCH], bf16, tag="wallT")
            nc.sync.dma_start(wallT[:], wallT_d[:])
            GSZ = 16
            strips = []
            for ci, (c0, h) in enumerate(CHUNKS):
                nt = len(_chunk_taps(c0, h))
                gts = []
                for gi, g0 in enumerate(range(0, nt, GSZ)):
                    gn = min(GSZ, nt - g0)
                    gt = cpool.tile([h, gn * h], bf16, tag=f"strip{ci}g{g0}",
                                    name=f"strip{ci}g{g0}")
                    eng = nc.sync if (ci + gi) % 2 == 0 else nc.scalar
                    eng.dma_start(gt[:], strip_d[ci][:, g0 * h:(g0 + gn) * h])
                    gts.append(gt)
                strips.append(gts)
            wsa = []
            for ci, (c0, h) in enumerate(CHUNKS):
                w = cpool.tile([h, SPD], f32, tag=f"wsa{ci}", name=f"wsa{ci}")
                nc.sync.dma_start(w[:], wsaT_d[c0:c0 + h, :])
                wsa.append(w)
            i100 = cpool.tile([SPD, SPD], f32, tag="i100")
            nc.sync.dma_start(i100[:], i100_d[:])
            ridge = cpool.tile([SPD, SPD], f32, tag="ridge")
            nc.sync.dma_start(ridge[:], ridge_d[:])
            iy = cpool.tile([SPD, SPD], f32, tag="iy")
            nc.sync.dma_start(iy[:], iy_d[:])
            msc = cpool.tile([SPD, SPD], f32, tag="msc")
            nc.sync.dma_start(msc[:], msc_d[:])
            ones_t = cpool.tile([TTW, 1], f32, tag="ones")
            nc.vector.memset(ones_t[:], 1.0)

            # ---- PE warmup: keep HAM busy while DMAs/LN1 land ----
            warm = ps_big.tile([128, 450], f32, tag="ps_big", name="ps_warm")
            for wi in range(24):
                nc.tensor.matmul(warm[:], wallT[:, 0:128], wallT[:],
                                 start=(wi == 0), stop=(wi == 23))

            def _ln(srcs, out, h, obase, ostep, dve_norm=False):
                st6 = spool.tile([h, 12], f32, tag="st6", name="st6")
                for hf in range(2):
                    nc.vector.bn_stats(st6[:, 6 * hf:6 * hf + 6], srcs[hf])
                mv = spool.tile([h, 2], f32, tag="mv", name="mv")
                nc.vector.bn_aggr(mv[:], st6[:])
                rs = spool.tile([h, 1], f32, tag="rs", name="rs")
                nc.vector.tensor_scalar(rs[:], mv[:, 1:2], EPS_LN, None, AL.add)
                nc.scalar.activation(rs[:], rs[:], AF.Sqrt)
                nc.vector.reciprocal(rs[:], rs[:])
                if dve_norm:
                    for hf in range(2):
                        nc.vector.tensor_scalar(
                            out[:, obase + hf * ostep:obase + hf * ostep + HALF],
                            srcs[hf], mv[:, 0:1], rs[:], AL.subtract, AL.mult)
                    return
                nb = spool.tile([h, 1], f32, tag="nb", name="nb")
                nc.vector.tensor_scalar(nb[:], mv[:, 0:1], rs[:], -1.0,
                                        AL.mult, AL.mult)
                for hf in range(2):
                    nc.scalar.activation(
                        out[:, obase + hf * ostep:obase + hf * ostep + HALF],
                        srcs[hf], AF.Identity, bias=nb[:], scale=rs[:])

            # ---- Phase A: all hpre + LN1 ----
            xhat_all, hcat_all, _XS = [], [], []
            for s in range(SPC):
                xs = spool.tile([CN, T], bf16, tag=f"xs{s}", name=f"xs{s}")
                nc.sync.dma_start(xs[:], x2_d[s])
                _XS.append(xs)
                xhat_all.append([wpool.tile([h, 2 * MARG + T], bf16,
                                            tag=f"xhat{ci}_{s}", name=f"xhat{ci}_{s}")
                                 for ci, (c0, h) in enumerate(CHUNKS)])
                hcat_all.append([wpool.tile([h, T], f32, tag=f"hcat{ci}_{s}",
                                            name=f"hcat{ci}_{s}")
                                 for ci, (c0, h) in enumerate(CHUNKS)])
                for ci, (c0, h) in enumerate(CHUNKS):
                    nc.gpsimd.memset(xhat_all[s][ci][:, 0:MARG], 0.0)
                    nc.gpsimd.memset(xhat_all[s][ci][:, MARG + T:], 0.0)
            # Phase A: hpre -> fast evict to SBUF (hcat buffer) -> LN1 -> xhat
            for s in range(SPC):
                for ci, (c0, h) in enumerate(CHUNKS):
                    hp = [ps_big.tile([h, HALF], f32, tag="ps_big", name="ps_hpre")
                          for _ in range(2)]
                    hsb = hcat_all[s][ci]
                    for hf in range(2):
                        nc.tensor.matmul(hp[hf][:], wallT[:, c0:c0 + h],
                                         _XS[s][:, hf * HALF:(hf + 1) * HALF])
                        nc.scalar.copy(hsb[:, hf * HALF:(hf + 1) * HALF],
                                       hp[hf][:])
                    _ln([hsb[:, 0:HALF], hsb[:, HALF:T]], xhat_all[s][ci],
                        h, MARG, HALF, dve_norm=True)

            # Phase B + per-sample posts
            haT_all = [[], []]
            ysb_all, b1_all, b2_all = [None, None], [None, None], [None, None]

            def _post(s):
                for j in range(NTT):
                    pt = ps_hat.tile([TTW, SPD], f32, tag="ps_hat", name="ps_hat")
                    for ci, (c0, h) in enumerate(CHUNKS):
                        nc.tensor.matmul(pt[:],
                                         hcat_all[s][ci][:, j * TTW:(j + 1) * TTW],
                                         wsa[ci][:], start=(ci == 0), stop=(ci == 3))
                    hj = spool.tile([TTW, SPD], f32, tag=f"haT{j}_{s}",
                                    name=f"haT{j}_{s}")
                    nc.scalar.copy(hj[:], pt[:])
                    haT_all[s].append(hj)
                haT = haT_all[s]
                gram = ps_sm.tile([SPD, SPD], f32, tag="ps_sm", name="gram")
                for j in range(NTT):
                    nc.tensor.matmul(gram[:], haT[j][:], haT[j][:],
                                     start=(j == 0), stop=(j == NTT - 1))
                mu = ps_sm.tile([1, SPD], f32, tag="ps_sm", name="mu")
                for j in range(NTT):
                    nc.tensor.matmul(mu[:], ones_t[:], haT[j][:],
                                     start=(j == 0), stop=(j == NTT - 1))
                s_sb = spool.tile([1, SPD], f32, tag=f"s_sb{s}", name=f"s_sb{s}")
                nc.scalar.copy(s_sb[:], mu[:])
                sst = ps_sm.tile([SPD, SPD], f32, tag="ps_sm", name="sst")
                nc.tensor.matmul(sst[:], s_sb[:], s_sb[:])
                tmp = spool.tile([SPD, SPD], f32, tag=f"ptmp{s}", name=f"ptmp{s}")
                nc.vector.scalar_tensor_tensor(
                    tmp[:], sst[:], -1.0 / (T * (T - 1.0)), ridge[:],
                    AL.mult, AL.add)
                psb = spool.tile([SPD, SPD], f32, tag=f"psb{s}", name=f"psb{s}")
                nc.vector.scalar_tensor_tensor(
                    psb[:], gram[:], 1.0 / (T - 1.0), tmp[:], AL.mult, AL.add)
                ysb = spool.tile([SPD, SPD], f32, tag=f"ysb{s}", name=f"ysb{s}")
                nc.vector.scalar_tensor_tensor(
                    ysb[:], psb[:], 2.0 / (CHEB_HI - CHEB_LO), iy[:],
                    AL.mult, AL.add)
                ysb_all[s] = ysb
                b1 = spool.tile([SPD, SPD], f32, tag=f"cl{s}_a", name=f"cl{s}_a")
                nc.vector.tensor_scalar(b1[:], i100[:], CC[CHEB_DEG], None, AL.mult)
                b2 = spool.tile([SPD, SPD], f32, tag=f"cl{s}_b", name=f"cl{s}_b")
                nc.vector.memset(b2[:], 0.0)
                b1_all[s] = b1; b2_all[s] = b2

            for s in range(SPC):
                for ci, (c0, h) in enumerate(CHUNKS):
                    taps = _chunk_taps(c0, h)
                    cv = [ps_big.tile([h, HALF], f32, tag="ps_big", name="ps_conv")
                          for _ in range(2)]
                    for u, dlt in enumerate(taps):
                        gt = strips[ci][u // 16]
                        uo = u % 16
                        for hf in range(2):
                            base = MARG + dlt + hf * HALF
                            nc.tensor.matmul(
                                cv[hf][:], gt[:, uo * h:(uo + 1) * h],
                                xhat_all[s][ci][:, base:base + HALF],
                                start=(u == 0), stop=(u == len(taps) - 1))
                    _ln([cv[0][:], cv[1][:]], hcat_all[s][ci], h, 0, HALF)
                _post(s)

            ni = [2, 2]
            names = [[f"cl{s}_a", f"cl{s}_b", f"cl{s}_c"] for s in range(SPC)]
            for j in range(CHEB_DEG - 1, -1, -1):
                for s in range(SPC):
                    yb = ps_sm.tile([SPD, SPD], f32, tag="ps_sm", name="yb")
                    nc.tensor.matmul(yb[:], ysb_all[s][:], b1_all[s][:])
                    cb = spool.tile([SPD, SPD], f32, tag=f"clcb{s}",
                                    name=f"clcb{s}")
                    nc.vector.scalar_tensor_tensor(
                        cb[:], i100[:], CC[j], b2_all[s][:], AL.mult, AL.subtract)
                    bk = spool.tile([SPD, SPD], f32, tag=names[s][ni[s]],
                                    name=names[s][ni[s]])
                    nc.vector.scalar_tensor_tensor(
                        bk[:], yb[:], 2.0 if j > 0 else 1.0, cb[:],
                        AL.mult, AL.add)
                    b2_all[s], b1_all[s] = b1_all[s], bk
                    ni[s] = (ni[s] + 1) % 3
            for s in range(SPC):
                lsc = spool.tile([SPD, SPD], f32, tag=f"lsc{s}", name=f"lsc{s}")
                nc.vector.tensor_tensor(lsc[:], b1_all[s][:], msc[:], AL.mult)
                nc.sync.dma_start(lout_d[s], lsc[:])

    nc.compile()
    return nc


def kernel(**inputs):
    global _PROG
    prep = _host_prep(inputs)
    if _PROG is None:
        _PROG = _build_program()
    nc = _PROG

    from concourse.bass_utils import run_bass_kernel_spmd
    x = np.asarray(inputs['x'], np.float32).astype(ml_dtypes.bfloat16)
    base = {
        'wallT': prep['wallT'], 'wsaT': prep['wsaT'], 'i100': prep['i100'],
        'ridge': prep['ridge'], 'iy': prep['iy'], 'mscale': prep['mscale'],
    }
    for ci in range(4):
        base[f'strip{ci}'] = prep['strips'][ci]
    in_maps = []
    for c in range(NCORES):
        m = dict(base)
        m['x2'] = np.ascontiguousarray(x[SPC * c:SPC * (c + 1)])
        in_maps.append(m)
    res = run_bass_kernel_spmd(nc, in_maps, list(range(NCORES))).results

    Ls = np.concatenate([res[c]['lout'] for c in range(NCORES)], 0)  # (16,100,100)
    iu = prep['iu']
    flat = Ls[:, iu[0], iu[1]].astype(np.float32)
    logits = flat @ prep['w_fc'].T + prep['b_fc']
    return logits, flat


# revision 17
# speedup vs baseline: 1.1152x; 1.1152x over previous
"""Trainium2 Bass kernel for nn_BaseModel_18081812316555.

Pipeline per sample (data-parallel over batch, 2 samples/core x 8 cores):
  hpre = Wall @ x            (Wall folds electrode conv + channel expand +
                              constrained depthwise spatial conv + pointwise)
  xhat = LN1(hpre)           -> bf16
  hcv  = depthwise temporal conv (PE diag-matmul accumulation, bf16->fp32 PSUM)
  hcat = LN2(hcv)
  h_aT = (w_aff @ w_sconv) @ hcat, built transposed (t on partitions)
  P    = gram(h_aT)/(T-1) - S S^T/(T(T-1)) + 1e-4 w_aff w_aff^T
  L    = logm(P) via Chebyshev/Clenshaw matrix polynomial on [0.05, 5.8]
  out  = L * vec-scale mask  (host extracts triu + final 4-way FC)
"""

import numpy as np
import ml_dtypes

B, CN, T = 16, 22, 1000
EXP, FCH, SPD, NCLS = 300, 150, 100, 4
KS = (15, 75, 55)
NCORES = 8
SPC = B // NCORES            # samples per core
CHEB_DEG = 32
CHEB_LO, CHEB_HI = 0.05, 5.8
EPS_LN = 1e-5

# channel chunking for the temporal conv (450 = 3 blocks x 150)
CHUNKS = [(0, 128), (128, 128), (256, 128), (384, 66)]   # (start, height)
HALF = 500                   # time-half width (2 x 500 = T)
MARG = 37                    # max K//2 across blocks


def _chan_K(ch):
    return KS[ch // FCH]


def _chunk_taps(c0, h):
    k2 = max(_chan_K(c0 + i) // 2 for i in range(h))
    return list(range(-k2, k2 + 1))


def _host_prep(inputs):
    d = {k: np.asarray(v, np.float64) for k, v in inputs.items()}
    wall, w5s, g_triv = [], [], True
    for bi, K in zip((1, 2, 3), KS):
        w1, w2 = d[f'b{bi}_w1'], d[f'b{bi}_w2']
        w3, w4 = d[f'b{bi}_w3'], d[f'b{bi}_w4']
        n = np.linalg.norm(w3, axis=1, keepdims=True)
        w3r = w3 * np.minimum(1.0, 1.0 / (n + 1e-12))
        weff = w4 @ (w2[:, None] * w3r)                      # (150,22)
        C = np.zeros((CN, CN))
        for i in range(CN):
            for j, k in enumerate((-1, 0, 1)):
                if 0 <= i + k < CN:
                    C[i, i + k] = w1[j]
        wall.append(weff @ C)
        w5s.append(d[f'b{bi}_w5'][:, 0, :])                  # (150,K)
        for nm in ('g1', 'g2'):
            if not np.allclose(d[f'b{bi}_{nm}'], 1.0):
                g_triv = False
        for nm in ('b1', 'b2'):
            if not np.allclose(d[f'b{bi}_{nm}'], 0.0):
                g_triv = False
    if not g_triv:
        raise NotImplementedError("non-trivial LN gain/bias not supported")
    wall = np.concatenate(wall, 0)                           # (450,22)
    w5cat = np.zeros((3 * FCH, max(KS)))
    for i, K in enumerate(KS):
        w5cat[FCH * i:FCH * (i + 1), :K] = w5s[i]

    # diag strips, bf16: strip[p, u*h + q] = w(chan, delta_u) * (p == q)
    strips = []
    for c0, h in CHUNKS:
        taps = _chunk_taps(c0, h)
        s = np.zeros((h, len(taps) * h), np.float64)
        for u, dlt in enumerate(taps):
            for p in range(h):
                K = _chan_K(c0 + p)
                k = dlt + K // 2
                if 0 <= k < K:
                    s[p, u * h + p] = w5cat[c0 + p, k]
        strips.append(s.astype(ml_dtypes.bfloat16))

    w_sa = d['w_aff'] @ d['w_sconv']                         # (100,450)
    cheb = np.polynomial.chebyshev.chebinterpolate(
        lambda y: np.log((CHEB_LO + CHEB_HI) / 2 + y * (CHEB_HI - CHEB_LO) / 2),
        CHEB_DEG)
    iu = np.triu_indices(SPD)
    msc = np.full((SPD, SPD), np.sqrt(2.0))
    np.fill_diagonal(msc, 1.0)

    c0_3, h_3 = CHUNKS[3]
    taps3 = _chunk_taps(c0_3, h_3)
    wcol3 = np.zeros((h_3, len(taps3)))
    for u, dlt in enumerate(taps3):
        for p in range(h_3):
            K = _chan_K(c0_3 + p)
            k = dlt + K // 2
            if 0 <= k < K:
                wcol3[p, u] = w5cat[c0_3 + p, k]

    f32 = np.float32
    return dict(
        wcol3=wcol3.astype(f32),
        wallT=np.ascontiguousarray(wall.T).astype(ml_dtypes.bfloat16),
        strips=strips,
        wsaT=np.ascontiguousarray(w_sa.T, f32),              # (450,100)
        i100=np.eye(SPD, dtype=f32),
        ridge=(1e-4 * (d['w_aff'] @ d['w_aff'].T)).astype(f32),
        iy=(-(CHEB_LO + CHEB_HI) / (CHEB_HI - CHEB_LO) * np.eye(SPD)).astype(f32),
        mscale=msc.astype(f32),
        cheb=[float(c) for c in cheb],
        w_fc=d['w_fc'].astype(f32), b_fc=d['b_fc'].astype(f32), iu=iu,
    )


_PROG = None


def _cheb_coeffs():
    c = np.polynomial.chebyshev.chebinterpolate(
        lambda y: np.log((CHEB_LO + CHEB_HI) / 2 + y * (CHEB_HI - CHEB_LO) / 2),
        CHEB_DEG)
    return [float(np.float32(v)) for v in c]


def _build_program():
    import concourse.bacc as bacc
    import concourse.tile as tile
    import concourse.mybir as mybir
    import concourse.bass as bass

    f32 = mybir.dt.float32
    bf16 = mybir.dt.bfloat16
    AL = mybir.AluOpType
    AF = mybir.ActivationFunctionType
    PSUM = bass.MemorySpace.PSUM

    nc = bacc.Bacc("TRN2", target_bir_lowering=False, debug=False)

    x2_d = nc.dram_tensor("x2", [SPC, CN, T], bf16, kind="ExternalInput")
    wallT_d = nc.dram_tensor("wallT", [CN, 3 * FCH], bf16, kind="ExternalInput")
    strip_d = []
    for ci, (c0, h) in enumerate(CHUNKS):
        nt = len(_chunk_taps(c0, h))
        strip_d.append(nc.dram_tensor(f"strip{ci}", [h, nt * h], bf16,
                                      kind="ExternalInput"))
    wsaT_d = nc.dram_tensor("wsaT", [3 * FCH, SPD], f32, kind="ExternalInput")
    i100_d = nc.dram_tensor("i100", [SPD, SPD], f32, kind="ExternalInput")
    ridge_d = nc.dram_tensor("ridge", [SPD, SPD], f32, kind="ExternalInput")
    iy_d = nc.dram_tensor("iy", [SPD, SPD], f32, kind="ExternalInput")
    msc_d = nc.dram_tensor("mscale", [SPD, SPD], f32, kind="ExternalInput")
    NT3 = len(_chunk_taps(*CHUNKS[3]))
    wcol3_d = nc.dram_tensor("wcol3", [CHUNKS[3][1], NT3], f32,
                             kind="ExternalInput")
    lout_d = nc.dram_tensor("lout", [SPC, SPD, SPD], f32, kind="ExternalOutput")

    CC = _cheb_coeffs()
    NTT = 8          # time tiles for the transposed h_a (8 x 125 = 1000)
    TTW = T // NTT

    with tile.TileContext(nc) as tc:
        with (
            tc.tile_pool(name="const", bufs=1) as cpool,
            tc.tile_pool(name="work", bufs=1) as wpool,
            tc.tile_pool(name="small", bufs=2) as spool,
            tc.tile_pool(name="ps_big", bufs=4, space=PSUM) as ps_big,
            tc.tile_pool(name="ps_hat", bufs=2, space=PSUM) as ps_hat,
            tc.tile_pool(name="ps_sm", bufs=2, space=PSUM) as ps_sm,
        ):
            # ---- constants / weights into SBUF ----
            wallT = cpool.tile([CN, 3 * FCH], bf16, tag="wallT")
            nc.sync.dma_start(wallT[:], wallT_d[:])
            GSZ = 16
            strips = []
            for ci, (c0, h) in enumerate(CHUNKS):
                nt = len(_chunk_taps(c0, h))
                gts = []
                for gi, g0 in enumerate(range(0, nt, GSZ)):
                    gn = min(GSZ, nt - g0)
                    gt = cpool.tile([h, gn * h], bf16, tag=f"strip{ci}g{g0}",
                                    name=f"strip{ci}g{g0}")
                    eng = nc.sync if (ci + gi) % 2 == 0 else nc.scalar
                    eng.dma_start(gt[:], strip_d[ci][:, g0 * h:(g0 + gn) * h])
                    gts.append(gt)
                strips.append(gts)
            wsa = []
            for ci, (c0, h) in enumerate(CHUNKS):
                w = cpool.tile([h, SPD], f32, tag=f"wsa{ci}", name=f"wsa{ci}")
                nc.sync.dma_start(w[:], wsaT_d[c0:c0 + h, :])
                wsa.append(w)
            i100 = cpool.tile([SPD, SPD], f32, tag="i100")
            nc.sync.dma_start(i100[:], i100_d[:])
            ridge = cpool.tile([SPD, SPD], f32, tag="ridge")
            nc.sync.dma_start(ridge[:], ridge_d[:])
            iy = cpool.tile([SPD, SPD], f32, tag="iy")
            nc.sync.dma_start(iy[:], iy_d[:])
            msc = cpool.tile([SPD, SPD], f32, tag="msc")
            nc.sync.dma_start(msc[:], msc_d[:])
            wcol3 = cpool.tile([CHUNKS[3][1], NT3], f32, tag="wcol3")
            nc.sync.dma_start(wcol3[:], wcol3_d[:])
            ones_t = cpool.tile([TTW, 1], f32, tag="ones")
            nc.vector.memset(ones_t[:], 1.0)

            # ---- PE warmup: keep HAM busy while DMAs/LN1 land ----
            warm = ps_big.tile([128, 450], f32, tag="ps_big", name="ps_warm")
            for wi in range(24):
                nc.tensor.matmul(warm[:], wallT[:, 0:128], wallT[:],
                                 start=(wi == 0), stop=(wi == 23))

            def _ln(srcs, out, h, obase, ostep, dve_norm=False):
                st6 = spool.tile([h, 12], f32, tag="st6", name="st6")
                for hf in range(2):
                    nc.vector.bn_stats(st6[:, 6 * hf:6 * hf + 6], srcs[hf])
                mv = spool.tile([h, 2], f32, tag="mv", name="mv")
                nc.vector.bn_aggr(mv[:], st6[:])
                rs = spool.tile([h, 1], f32, tag="rs", name="rs")
                nc.vector.tensor_scalar(rs[:], mv[:, 1:2], EPS_LN, None, AL.add)
                nc.scalar.activation(rs[:], rs[:], AF.Sqrt)
                nc.vector.reciprocal(rs[:], rs[:])
                if dve_norm:
                    for hf in range(2):
                        nc.vector.tensor_scalar(
                            out[:, obase + hf * ostep:obase + hf * ostep + HALF],
                            srcs[hf], mv[:, 0:1], rs[:], AL.subtract, AL.mult)
                    return
                nb = spool.tile([h, 1], f32, tag="nb", name="nb")
                nc.vector.tensor_scalar(nb[:], mv[:, 0:1], rs[:], -1.0,
                                        AL.mult, AL.mult)
                for hf in range(2):
                    nc.scalar.activation(
                        out[:, obase + hf * ostep:obase + hf * ostep + HALF],
                        srcs[hf], AF.Identity, bias=nb[:], scale=rs[:])

            # ---- Phase A: all hpre + LN1 ----
            xhat_all, hcat_all, _XS = [], [], []
            for s in range(SPC):
                xs = spool.tile([CN, T], bf16, tag=f"xs{s}", name=f"xs{s}")
                nc.sync.dma_start(xs[:], x2_d[s])
                _XS.append(xs)
                xhat_all.append([wpool.tile([h, 2 * MARG + T], bf16,
                                            tag=f"xhat{ci}_{s}", name=f"xhat{ci}_{s}")
                                 for ci, (c0, h) in enumerate(CHUNKS)])
                hcat_all.append([wpool.tile([h, T], f32, tag=f"hcat{ci}_{s}",
                                            name=f"hcat{ci}_{s}")
                                 for ci, (c0, h) in enumerate(CHUNKS)])
                for ci, (c0, h) in enumerate(CHUNKS):
                    nc.gpsimd.memset(xhat_all[s][ci][:, 0:MARG], 0.0)
                    nc.gpsimd.memset(xhat_all[s][ci][:, MARG + T:], 0.0)
            # Phase A: hpre -> fast evict to SBUF (hcat buffer) -> LN1 -> xhat
            for s in range(SPC):
                for ci, (c0, h) in enumerate(CHUNKS):
                    hp = [ps_big.tile([h, HALF], f32, tag="ps_big", name="ps_hpre")
                          for _ in range(2)]
                    hsb = hcat_all[s][ci]
                    for hf in range(2):
                        nc.tensor.matmul(hp[hf][:], wallT[:, c0:c0 + h],
                                         _XS[s][:, hf * HALF:(hf + 1) * HALF])
                        nc.scalar.copy(hsb[:, hf * HALF:(hf + 1) * HALF],
                                       hp[hf][:])
                    _ln([hsb[:, 0:HALF], hsb[:, HALF:T]], xhat_all[s][ci],
                        h, MARG, HALF, dve_norm=True)

            # Phase B + per-sample posts
            haT_all = [[], []]
            ysb_all, b1_all, b2_all = [None, None], [None, None], [None, None]

            def _post(s):
                for j in range(NTT):
                    pt = ps_hat.tile([TTW, SPD], f32, tag="ps_hat", name="ps_hat")
                    for ci, (c0, h) in enumerate(CHUNKS):
                        nc.tensor.matmul(pt[:],
                                         hcat_all[s][ci][:, j * TTW:(j + 1) * TTW],
                                         wsa[ci][:], start=(ci == 0), stop=(ci == 3))
                    hj = spool.tile([TTW, SPD], f32, tag=f"haT{j}_{s}",
                                    name=f"haT{j}_{s}")
                    nc.scalar.copy(hj[:], pt[:])
                    haT_all[s].append(hj)
                haT = haT_all[s]
                gram = ps_sm.tile([SPD, SPD], f32, tag="ps_sm", name="gram")
                for j in range(NTT):
                    nc.tensor.matmul(gram[:], haT[j][:], haT[j][:],
                                     start=(j == 0), stop=(j == NTT - 1))
                mu = ps_sm.tile([1, SPD], f32, tag="ps_sm", name="mu")
                for j in range(NTT):
                    nc.tensor.matmul(mu[:], ones_t[:], haT[j][:],
                                     start=(j == 0), stop=(j == NTT - 1))
                s_sb = spool.tile([1, SPD], f32, tag=f"s_sb{s}", name=f"s_sb{s}")
                nc.scalar.copy(s_sb[:], mu[:])
                sst = ps_sm.tile([SPD, SPD], f32, tag="ps_sm", name="sst")
                nc.tensor.matmul(sst[:], s_sb[:], s_sb[:])
                tmp = spool.tile([SPD, SPD], f32, tag=f"ptmp{s}", name=f"ptmp{s}")
                nc.vector.scalar_tensor_tensor(
                    tmp[:], sst[:], -1.0 / (T * (T - 1.0)), ridge[:],
                    AL.mult, AL.add)
                psb = spool.tile([SPD, SPD], f32, tag=f"psb{s}", name=f"psb{s}")
                nc.vector.scalar_tensor_tensor(
                    psb[:], gram[:], 1.0 / (T - 1.0), tmp[:], AL.mult, AL.add)
                ysb = spool.tile([SPD, SPD], f32, tag=f"ysb{s}", name=f"ysb{s}")
                nc.vector.scalar_tensor_tensor(
                    ysb[:], psb[:], 2.0 / (CHEB_HI - CHEB_LO), iy[:],
                    AL.mult, AL.add)
                ysb_all[s] = ysb
                b1 = spool.tile([SPD, SPD], f32, tag=f"cl{s}_a", name=f"cl{s}_a")
                nc.vector.tensor_scalar(b1[:], i100[:], CC[CHEB_DEG], None, AL.mult)
                b2 = spool.tile([SPD, SPD], f32, tag=f"cl{s}_b", name=f"cl{s}_b")
                nc.vector.memset(b2[:], 0.0)
                b1_all[s] = b1; b2_all[s] = b2

            for s in range(SPC):
                for ci, (c0, h) in enumerate(CHUNKS):
                    taps = _chunk_taps(c0, h)
                    if ci == 3:
                        # DVE path: scalar_tensor_tensor accumulation
                        acc = wpool.tile([h, T], f32, tag=f"acc3_{s}",
                                         name=f"acc3_{s}")
                        for u, dlt in enumerate(taps):
                            for hf in range(2):
                                base = MARG + dlt + hf * HALF
                                xsl = xhat_all[s][ci][:, base:base + HALF]
                                osl = acc[:, hf * HALF:(hf + 1) * HALF]
                                if u == 0:
                                    nc.vector.tensor_scalar(
                                        osl, xsl, wcol3[:, u:u + 1], None,
                                        AL.mult)
                                else:
                                    nc.vector.scalar_tensor_tensor(
                                        osl, xsl, wcol3[:, u:u + 1], osl,
                                        AL.mult, AL.add)
                        _ln([acc[:, 0:HALF], acc[:, HALF:T]],
                            hcat_all[s][ci], h, 0, HALF)
                        continue
                    cv = [ps_big.tile([h, HALF], f32, tag="ps_big", name="ps_conv")
                          for _ in range(2)]
                    for u, dlt in enumerate(taps):
                        gt = strips[ci][u // 16]
                        uo = u % 16
                        for hf in range(2):
                            base = MARG + dlt + hf * HALF
                            nc.tensor.matmul(
                                cv[hf][:], gt[:, uo * h:(uo + 1) * h],
                                xhat_all[s][ci][:, base:base + HALF],
                                start=(u == 0), stop=(u == len(taps) - 1))
                    _ln([cv[0][:], cv[1][:]], hcat_all[s][ci], h, 0, HALF)
                _post(s)

            ni = [2, 2]
            names = [[f"cl{s}_a", f"cl{s}_b", f"cl{s}_c"] for s in range(SPC)]
            for j in range(CHEB_DEG - 1, -1, -1):
                for s in range(SPC):
                    yb = ps_sm.tile([SPD, SPD], f32, tag="ps_sm", name="yb")
                    nc.tensor.matmul(yb[:], ysb_all[s][:], b1_all[s][:])
                    cb = spool.tile([SPD, SPD], f32, tag=f"clcb{s}",
                                    name=f"clcb{s}")
                    nc.vector.scalar_tensor_tensor(
                        cb[:], i100[:], CC[j], b2_all[s][:], AL.mult, AL.subtract)
                    bk = spool.tile([SPD, SPD], f32, tag=names[s][ni[s]],
                                    name=names[s][ni[s]])
                    nc.vector.scalar_tensor_tensor(
                        bk[:], yb[:], 2.0 if j > 0 else 1.0, cb[:],
                        AL.mult, AL.add)
                    b2_all[s], b1_all[s] = b1_all[s], bk
                    ni[s] = (ni[s] + 1) % 3
            for s in range(SPC):
                lsc = spool.tile([SPD, SPD], f32, tag=f"lsc{s}", name=f"lsc{s}")
                nc.vector.tensor_tensor(lsc[:], b1_all[s][:], msc[:], AL.mult)
                nc.sync.dma_start(lout_d[s], lsc[:])

    nc.compile()
    return nc


def kernel(**inputs):
    global _PROG
    prep = _host_prep(inputs)
    if _PROG is None:
        _PROG = _build_program()
    nc = _PROG

    from concourse.bass_utils import run_bass_kernel_spmd
    x = np.asarray(inputs['x'], np.float32).astype(ml_dtypes.bfloat16)
    base = {
        'wcol3': prep['wcol3'],
        'wallT': prep['wallT'], 'wsaT': prep['wsaT'], 'i100': prep['i100'],
        'ridge': prep['ridge'], 'iy': prep['iy'], 'mscale': prep['mscale'],
    }
    for ci in range(4):
        base[f'strip{ci}'] = prep['strips'][ci]
    in_maps = []
    for c in range(NCORES):
        m = dict(base)
        m['x2'] = np.ascontiguousarray(x[SPC * c:SPC * (c + 1)])
        in_maps.append(m)
    res = run_bass_kernel_spmd(nc, in_maps, list(range(NCORES))).results

    Ls = np.concatenate([res[c]['lout'] for c in range(NCORES)], 0)  # (16,100,100)
    iu = prep['iu']
    flat = Ls[:, iu[0], iu[1]].astype(np.float32)
    logits = flat @ prep['w_fc'].T + prep['b_fc']
    return logits, flat
